# revision 34
# baseline (speedup 1.0000x reference)
"""NeRF render kernel for 8 Trainium2 NeuronCores.

Data-parallel over rays: core k handles rays [2048*k, 2048*(k+1)).
Per core: positional encoding + 3-layer MLP (39->256->256->4) over
131072 points in feature-major layout (features on partitions, points
on the free dim), then alpha compositing via triangular-matrix matmul
cumulative sums.

Point permutation inside a core: t_rand rows are loaded as
[128 partitions = ray-group i (rays 16i..16i+15), 1024 = (k, s)] and
PE-transposed per 128-column chunk k0 so that partitions become
q = rp*64 + s (rp = ray parity) and columns J = 128*k0 + i denote the
ray pair (16i + 2*k0, 16i + 2*k0 + 1).  All downstream tiles keep that
column order; the host unscatters at the end.

Performance layout (v11):
- Staging is (block, feature64) so each 2-block pair is a contiguous
  [128, 128] unit; PE transposes move it to 64-aligned partition
  bands, and the L0 matmuls run row-tiled band pairs concurrently.
- The whole MLP is software-pipelined over 128 super-tiles: at
  iteration it the PE runs transposes(it), L0(it-1), L1(it-2),
  L2(it-3), so it never stalls on the ACT/DVE PSUM drains and the
  HAM clock stays warm (2.4 GHz).
- PSUM->SBUF relu drains split evenly ACT/DVE (h0 drains paired as
  single FD-1024 ops); sin-arg chain tensor_scalars on DVE/ACT,
  tensor_tensors on GPSIMD; cos features via cos(2a) = 1-2 sin^2(a)
  in fp16; t_rand is transposed host-side; compositing is deferred
  to an epilogue with all sigmoids batched before all exps (one
  activation-table swap each).
"""

import sys
import numpy as np

sys.path.insert(0, "/opt/trn_rl_repo")

S = 64
L = 6
NCORES = 8
B = 16384
BC = B // NCORES          # rays per core
NP = BC * S               # points per core
NBLK = NP // 128          # 1024 ray-pair blocks
NGRP = 8                  # groups of 128 blocks
HB = 64                   # blocks per half-group
NEAR, FAR = 2.0, 6.0
DELTA = (FAR - NEAR) / S
PI = float(np.pi)
TWO_PI = float(2.0 * np.pi)
INV2PI = float(np.float32(1.0 / (2.0 * np.pi)))
MAGIC = 12582912.0  # 1.5 * 2**23: float32 round-to-int trick
C1 = float(np.float32(2.0 * np.pi))
C2 = float(2.0 * np.pi - np.float64(np.float32(2.0 * np.pi)))

_CACHE = {}
PROFILE = False  # test harness sets True to collect an NTFF trace


def _split_waits(nc, mybir):
    """TRN2 allows one sem wait per instruction (two for EventSemaphore);
    this walrus build rejects over-limit instructions, so move excess waits
    onto chained NOPs on the same engine just before the instruction."""
    ctr = 0
    for fn in nc.m.functions:
        for bb in fn.blocks:
            changed = False
            out = []
            for inst in bb.instructions:
                si = inst.sync_info
                cap = 2 if isinstance(inst, mybir.InstEventSemaphore) else 1
                if si is not None and si.on_wait and len(si.on_wait) > cap:
                    waits = list(si.on_wait)
                    for w in waits[:-cap]:
                        nop = mybir.InstNoOp(
                            name=f"wsplit-{ctr}", ins=[], outs=[]
                        )
                        ctr += 1
                        nop.engine = inst.engine
                        nop.sync_info = mybir.SyncInfo(on_wait=[w], on_update=[])
                        nc.register_instruction(nop)
                        out.append(nop)
                    si.on_wait = waits[-cap:]
                    changed = True
                out.append(inst)
            if changed:
                bb.instructions = out
    return ctr


def _build():
    import concourse.bass as bass
    import concourse.mybir as mybir
    import concourse.tile as tile

    dt = mybir.dt
    AF = mybir.ActivationFunctionType
    OP = mybir.AluOpType
    F32 = dt.float32
    F32R = dt.float32r
    F16 = dt.float16

    nc = bass.Bass()

    # ---- DRAM I/O ----
    tnat_d = nc.dram_tensor("tnat", [128, 1024], F32, kind="ExternalInput")
    aexp_d = nc.dram_tensor("aexp", [3, 128, 1024], F32, kind="ExternalInput")
    bexp_d = nc.dram_tensor("bexp", [3, 128, 1024], F32, kind="ExternalInput")
    w0_d = nc.dram_tensor("w0rep", [128, 256], F16, kind="ExternalInput")
    w1_d = nc.dram_tensor("w1", [256, 256], F16, kind="ExternalInput")
    w2_d = nc.dram_tensor("w2h", [128, 8], F16, kind="ExternalInput")
    b0_d = nc.dram_tensor("b0t", [128, 2], F32, kind="ExternalInput")
    b1_d = nc.dram_tensor("b1t", [128, 2], F32, kind="ExternalInput")
    b2_d = nc.dram_tensor("b2t", [128, 4], F32, kind="ExternalInput")
    zcpp_d = nc.dram_tensor("zcpp", [128, 1], F32, kind="ExternalInput")
    ltri_d = nc.dram_tensor("ltri", [128, 256], F32, kind="ExternalInput")
    sel2_d = nc.dram_tensor("sel2", [128, 2], F32R, kind="ExternalInput")
    ident_d = nc.dram_tensor("ident", [128, 128], F32R, kind="ExternalInput")
    identh_d = nc.dram_tensor("identh", [128, 128], F16, kind="ExternalInput")
    out_d = nc.dram_tensor("out", [NGRP, 2, 384], F32, kind="ExternalOutput")

    with tile.TileContext(nc) as tc:
        with (
            tc.tile_pool(name="consts", bufs=1) as cpool,
            tc.tile_pool(name="tall", bufs=1) as tpool,
            tc.tile_pool(name="o2", bufs=8) as o2pool,
        ):
            # ---- load constants / weights ----
            tnat = cpool.tile([128, 1024], F32, tag="tnat")
            nc.sync.dma_start(tnat[:], tnat_d[:])
            w0rep = cpool.tile([128, 256], F16, tag="w0rep")
            nc.sync.dma_start(w0rep[:], w0_d[:])
            w1s0 = cpool.tile([128, 256], F16, tag="w1s0")
            nc.sync.dma_start(w1s0[:], w1_d[0:128, :])
            w1s1 = cpool.tile([128, 256], F16, tag="w1s1")
            nc.sync.dma_start(w1s1[:], w1_d[128:256, :])
            w2s = cpool.tile([128, 8], F16, tag="w2s")
            nc.sync.dma_start(w2s[:], w2_d[:])
            b0t = cpool.tile([128, 2], F32, tag="b0t")
            nc.sync.dma_start(b0t[:], b0_d[:])
            b1t = cpool.tile([128, 2], F32, tag="b1t")
            nc.sync.dma_start(b1t[:], b1_d[:])
            b2t = cpool.tile([128, 4], F32, tag="b2t")
            nc.sync.dma_start(b2t[:], b2_d[:])
            zcpp = cpool.tile([128, 1], F32, tag="zcpp")
            nc.sync.dma_start(zcpp[:], zcpp_d[:])
            ltri = cpool.tile([128, 256], F32, tag="ltri")
            nc.sync.dma_start(ltri[:], ltri_d[:])
            sel2 = cpool.tile([128, 2], F32R, tag="sel2")
            nc.sync.dma_start(sel2[:], sel2_d[:])
            ident = cpool.tile([128, 128], F32R, tag="ident")
            nc.sync.dma_start(ident[:], ident_d[:])
            identh = cpool.tile([128, 128], F16, tag="identh")
            nc.sync.dma_start(identh[:], identh_d[:])
            zerot = cpool.tile([128, 1], F32, tag="zerot")
            nc.vector.memset(zerot[:], 0.0)
            magict = cpool.tile([128, 1], F32, tag="magict")
            nc.vector.memset(magict[:], MAGIC)

            # ---- phase A: tnat arrives host-transposed; z + pts +
            # range reduction are all computed per half-group inside
            # chain_gen, so the serial head is just the DMA loads ----
            zt = tpool.tile([128, 1024], F32, tag="zt")
            pts3 = tpool.tile([128, 3072], F32, tag="pts3")
            pts3r = tpool.tile([128, 3072], F32, tag="pts3r")
            if True:
                aexs = []
                bexs = []
                for c in range(3):
                    ae = cpool.tile([128, 1024], F32, tag=f"aex{c}", name=f"aex{c}")
                    nc.sync.dma_start(ae[:], aexp_d[c])
                    be = cpool.tile([128, 1024], F32, tag=f"bex{c}", name=f"bex{c}")
                    nc.sync.dma_start(be[:], bexp_d[c])
                    aexs.append(ae)
                    bexs.append(be)

            # ---- phase B + C: software-pipelined over 128 super-tiles ----
            # At iteration it the PE runs transposes(it), L0(it-1),
            # L1(it-2), L2(it-3): every stage consumes activations that
            # were drained a full iteration earlier, so the PE never
            # stalls on ACT/DVE drains and the HAM clock stays warm.
            NS = NBLK // 8            # 128 supers; super s = blocks 8s..8s+7
            NHG = 2 * NGRP            # 16 half-groups of 8 supers
            W = 3 * HB
            with (
                tc.tile_pool(name="scr", bufs=2) as scrpool,
                tc.tile_pool(name="kpool", bufs=2) as kpool,
                tc.tile_pool(name="sfp", bufs=2) as sfppool,
                tc.tile_pool(name="fs", bufs=4) as fspool,
                tc.tile_pool(name="h0s", bufs=4) as h0spool,
                tc.tile_pool(name="h1s", bufs=4) as h1spool,
                tc.tile_pool(name="cS", bufs=2) as cspool,
                tc.tile_pool(name="tpP", bufs=1, space="PSUM") as tp_pool,
                tc.tile_pool(name="h0P", bufs=2, space="PSUM") as h0_pool,
                tc.tile_pool(name="h1P", bufs=2, space="PSUM") as h1_pool,
                tc.tile_pool(name="oP", bufs=1, space="PSUM") as o_pool,
            ):
                sf_t = {}
                fs_t = {}
                h0_t = {}
                h1_t = {}
                og_t = {}

                def chain_gen(hg):
                    """Sin-arg chain + staging for one half-group, split
                    into 8 steps (one per pipeline iteration).

                    sa (f, j): rows 0:3 = x/2, 3:6 = x, 3(l+1)+c = wrapped
                    2^l x.  sf (j, f64): per block j a 64-slot group
                    [0:18 sin, 18:36 cos, 36:39 raw, 39:42 sin(x/2),
                    42:64 pad] so each 2-block pair is a contiguous
                    [128, 128] unit that a REGULAR matmul (chunk
                    stationary + FWL, identity moving) transposes into
                    64-aligned bands -- counting as PE activity (warm HAM).
                    """
                    jsl = slice(HB * hg, HB * (hg + 1))
                    csl = slice(3 * HB * hg, 3 * HB * (hg + 1))
                    nc.vector.tensor_scalar(
                        zt[:, jsl], tnat[:, jsl], DELTA, zcpp[:, 0:1],
                        op0=OP.mult, op1=OP.add,
                    )
                    for c in range(3):
                        pv = pts3.rearrange("p (j c) -> p c j", c=3)[:, c, jsl]
                        nc.vector.tensor_tensor(
                            pv, zt[:, jsl], bexs[c][:, jsl], op=OP.mult
                        )
                        nc.vector.tensor_tensor(
                            pv, pv, aexs[c][:, jsl], op=OP.add
                        )
                    kt = kpool.tile([128, W], F32, tag="kt", name="kt")
                    nc.vector.tensor_scalar(
                        kt[:], pts3[:, csl], INV2PI, MAGIC, op0=OP.mult, op1=OP.add
                    )
                    nc.vector.tensor_scalar(
                        kt[:], kt[:], MAGIC, -C1, op0=OP.subtract, op1=OP.mult
                    )
                    nc.vector.tensor_tensor(
                        pts3r[:, csl], pts3[:, csl], kt[:], op=OP.add
                    )
                    nc.vector.tensor_scalar(
                        pts3r[:, csl], pts3r[:, csl], PI, -PI, op0=OP.min, op1=OP.max
                    )
                    yield
                    sa = scrpool.tile([128, 7 * W], F32, tag="sa", name="sa")
                    p3rt = pts3r.rearrange(
                        "p (o j c) -> p o c j", o=NHG, c=3
                    )[:, hg, :, :]
                    sa7 = sa.rearrange("p (f c j) -> p f c j", f=7, c=3)
                    nc.vector.tensor_scalar(
                        sa7[:, 0], p3rt, 0.5, None, op0=OP.mult
                    )
                    nc.gpsimd.tensor_copy(sa7[:, 1], p3rt)
                    # r_l = 2 r_{l-1} - 2pi*round(r_{l-1}/pi)
                    for l in range(1, L):
                        prev = sa[:, l * W : (l + 1) * W]
                        cur = sa[:, (l + 1) * W : (l + 2) * W]
                        kb = kpool.tile([128, W], F32, tag="kb", name="kb")
                        nc.scalar.activation(
                            kb[:], prev, AF.Identity,
                            scale=2.0 * INV2PI, bias=magict[:, 0:1],
                        )
                        nc.vector.tensor_scalar(
                            kb[:], kb[:], MAGIC, -PI,
                            op0=OP.subtract, op1=OP.mult,
                        )
                        nc.gpsimd.tensor_tensor(cur, prev, kb[:], op=OP.add)
                        nc.vector.tensor_scalar(
                            cur, cur, 2.0, None, op0=OP.mult
                        )
                        if l < L - 1:
                            yield
                    sf = sfppool.tile([128, 64 * HB], F16, tag="sf", name="sf")
                    sf_t[hg] = sf
                    sfv = sf.rearrange("p (j f) -> p j f", f=64)
                    saj = sa.rearrange("p (f j) -> p j f", j=HB)
                    nc.scalar.activation(
                        sfv[:, :, 39:42], saj[:, :, 0:3], AF.Sin
                    )
                    yield
                    nc.scalar.activation(
                        sfv[:, :, 0:18], saj[:, :, 3:21], AF.Sin
                    )
                    # cos_l = 1 - 2 sin(2^{l-1} x)^2   (fp16, DVE)
                    for l in range(L):
                        if l == 2:
                            yield
                        src = (sfv[:, :, 39:42] if l == 0
                               else sfv[:, :, 3 * (l - 1) : 3 * l])
                        kb16 = kpool.tile([128, W], F16, tag="kb16", name="kb16")
                        kb16v = kb16.rearrange("p (j c) -> p j c", c=3)
                        nc.vector.tensor_tensor(kb16v, src, src, op=OP.mult)
                        nc.vector.tensor_scalar(
                            sfv[:, :, 18 + 3 * l : 21 + 3 * l],
                            kb16v, -2.0, 1.0, op0=OP.mult, op1=OP.add,
                        )
                    nc.vector.tensor_copy(
                        sfv[:, :, 36:39],
                        pts3.rearrange("p (j c) -> p j c", c=3)[
                            :, HB * hg : HB * (hg + 1), :
                        ],
                    )
                    yield

                def stage_T(s):
                    sf = sf_t[s // 8]
                    stl = s % 8
                    tp = tp_pool.tile([128, 512], F16, tag="tp", name="tp")
                    for k in range(4):
                        jj = 4 * stl + k
                        nc.tensor.transpose(
                            tp[:, 128 * k : 128 * (k + 1)],
                            sf[:, 128 * jj : 128 * (jj + 1)],
                            identh[:],
                        )
                    fs = fspool.tile([128, 512], F16, tag="fs", name="fs")
                    if s % 4 == 0:
                        nc.vector.tensor_copy(fs[:], tp[:])
                    else:
                        nc.scalar.activation(fs[:], tp[:], AF.Copy)
                    fs_t[s] = fs

                def stage_L0(s):
                    # L0 row-tiled band pairs; per hidden-half the two
                    # band outputs share one [128, 1024] PSUM tile so the
                    # relu+bias drain is a single FD-1024 op (uniform bias)
                    fs = fs_t.pop(s)
                    h0ss = [
                        h0spool.tile([128, 1024], F16, tag="h0s", name=f"h0s{s}_{h}")
                        for h in range(2)
                    ]
                    for h in range(2):
                        h0p = h0_pool.tile([128, 1024], F32, tag="h0p", name="h0p")
                        for x in range(2):
                            lo = 64 * x
                            nc.tensor.matmul(
                                h0p[:, 512 * x : 512 * (x + 1)],
                                w0rep[lo : lo + 39, 128 * h : 128 * (h + 1)],
                                fs[lo : lo + 39, :],
                            )
                        if h == 0:
                            nc.scalar.activation(
                                h0ss[0][:], h0p[:], AF.Relu, bias=b0t[:, 0:1]
                            )
                        else:
                            nc.vector.tensor_scalar(
                                h0ss[1][:], h0p[:], b0t[:, 1:2], 0.0,
                                op0=OP.add, op1=OP.max,
                            )
                    h0_t[s] = h0ss

                def stage_L1(s):
                    h0ss = h0_t.pop(s)
                    h1ss = [
                        h1spool.tile([128, 1024], F16, tag="h1s", name=f"h1s{s}_{g}")
                        for g in range(2)
                    ]
                    for x in range(2):
                        for gh in range(2):
                            h1p = h1_pool.tile([128, 512], F32, tag="h1p", name="h1p")
                            nc.tensor.matmul(
                                h1p[:],
                                w1s0[:, 128 * gh : 128 * (gh + 1)],
                                h0ss[0][:, 512 * x : 512 * (x + 1)],
                                start=True,
                                stop=False,
                            )
                            nc.tensor.matmul(
                                h1p[:],
                                w1s1[:, 128 * gh : 128 * (gh + 1)],
                                h0ss[1][:, 512 * x : 512 * (x + 1)],
                                start=False,
                                stop=True,
                            )
                            dst = h1ss[gh][:, 512 * x : 512 * (x + 1)]
                            if gh == 0:
                                nc.scalar.activation(
                                    dst, h1p[:], AF.Relu, bias=b1t[:, 0:1]
                                )
                            else:
                                nc.vector.tensor_scalar(
                                    dst, h1p[:], b1t[:, 1:2], 0.0,
                                    op0=OP.add, op1=OP.max,
                                )
                    h1_t[s] = h1ss

                def stage_L2(s):
                    h1ss = h1_t.pop(s)
                    g = s // 16
                    if s % 16 == 0:
                        og_t[g] = o_pool.tile([128, 512], F32, tag="og", name="og")
                    og = og_t[g]
                    # band x holds blocks of parity x
                    for x in range(2):
                        for jp in range(4):
                            jj = 8 * (s % 16) + 2 * jp + x
                            nc.tensor.matmul(
                                og[:, 4 * jj : 4 * (jj + 1)],
                                h1ss[0][:, 512 * x + 128 * jp : 512 * x + 128 * (jp + 1)],
                                w2s[:, 0:4],
                                start=True,
                                stop=False,
                            )
                            nc.tensor.matmul(
                                og[:, 4 * jj : 4 * (jj + 1)],
                                h1ss[1][:, 512 * x + 128 * jp : 512 * x + 128 * (jp + 1)],
                                w2s[:, 4:8],
                                start=False,
                                stop=True,
                            )
                    if s % 16 == 15:
                        emit_groupC(g)

                o2_t = {}

                def emit_groupC(g):
                    # og -> o2 drain only (no table-switching ACT funcs);
                    # the compositing itself is deferred to the epilogue
                    og = og_t.pop(g)
                    o2 = o2pool.tile([128, 512], F32, tag="o2", name="o2")
                    o2_t[g] = o2
                    ogv = og.rearrange("p (j c) -> p j c", c=4)
                    o2v = o2.rearrange("p (j c) -> p j c", c=4)
                    nc.scalar.activation(
                        o2v[:, :, 0], ogv[:, :, 0], AF.Identity, bias=b2t[:, 0:1]
                    )
                    nc.vector.tensor_scalar(
                        o2v[:, :, 1], ogv[:, :, 1], b2t[:, 1:2], None, op0=OP.add
                    )
                    nc.scalar.activation(
                        o2v[:, :, 2], ogv[:, :, 2], AF.Identity, bias=b2t[:, 2:3]
                    )
                    nc.vector.tensor_scalar(
                        o2v[:, :, 3], ogv[:, :, 3], b2t[:, 3:4], 0.0,
                        op0=OP.add, op1=OP.max,
                    )

                e_t = {}

                def emit_sigmoid(g):
                    o2v = o2_t[g].rearrange("p (j c) -> p j c", c=4)
                    e = cspool.tile([128, 384], F32, tag="e", name="e", bufs=NGRP)
                    e_t[g] = e
                    nc.scalar.activation(
                        e.rearrange("p (j c) -> p j c", c=3),
                        o2v[:, :, 0:3],
                        AF.Sigmoid,
                    )

                def emit_compositing(g):
                    o2 = o2_t.pop(g)
                    o2v = o2.rearrange("p (j c) -> p j c", c=4)
                    e = e_t.pop(g)
                    # scans: exclusive & inclusive cumsum of sigma over s
                    ct = h1_pool.tile([128, 512], F32, tag="h1p", name="ct")
                    sig = o2v[:, :, 3]
                    nc.tensor.matmul(ct[:, 0:128], ltri[:, 0:128], sig)
                    nc.tensor.matmul(ct[:, 128:256], ltri[:, 128:256], sig)
                    texin = cspool.tile([128, 256], F32, tag="texin", name="texin")
                    nc.scalar.activation(texin[:], ct[:, 0:256], AF.Exp, scale=-DELTA)
                    wt = cspool.tile([128, 128], F32, tag="wt", name="wt")
                    nc.gpsimd.tensor_tensor(
                        wt[:], texin[:, 0:128], texin[:, 128:256], op=OP.subtract
                    )
                    wr = cspool.tile([128, 384], F32R, tag="wr", name="wr")
                    nc.gpsimd.tensor_tensor(
                        wr.rearrange("p (j c) -> p j c", c=3),
                        e.rearrange("p (j c) -> p j c", c=3),
                        wt.unsqueeze(2).broadcast_to([128, 128, 3]),
                        op=OP.mult,
                    )
                    # final per-ray-parity sum into spare cols of ct's bank
                    rp_ = ct[0:2, 128:512]
                    nc.tensor.matmul(rp_, sel2[:], wr[:])
                    outs = cspool.tile([2, 384], F32, tag="outs", name="outs")
                    nc.vector.tensor_copy(outs[:], rp_)
                    nc.sync.dma_start(out_d[g], outs[:])

                gen = chain_gen(0)
                for _ in gen:
                    pass
                gens = {}
                for it in range(NS + 3):
                    if it < NS:
                        hg_next = it // 8 + 1
                        if hg_next < NHG:
                            if it % 8 == 0:
                                gens[hg_next] = chain_gen(hg_next)
                            next(gens[hg_next], None)
                        stage_T(it)
                    if 1 <= it <= NS:
                        stage_L0(it - 1)
                    if 2 <= it <= NS + 1:
                        stage_L1(it - 2)
                    if 3 <= it <= NS + 2:
                        stage_L2(it - 3)
                for g in range(NGRP):
                    emit_sigmoid(g)
                for g in range(NGRP):
                    emit_compositing(g)

    _split_waits(nc, mybir)
    return nc


def _host_prep(origins, directions, t_rand, W0, b0, W1, b1, W2, b2):
    """Build per-core input maps (all numpy, cheap)."""
    f32 = np.float32
    # F-row order: rows 3l+c = sin freq l coord c; 18+3l+c = cos; 36..38 pts
    perm = np.zeros(39, np.int64)
    perm[36:39] = (0, 1, 2)
    for l in range(L):
        for c in range(3):
            perm[3 * l + c] = 3 + 6 * l + c
            perm[18 + 3 * l + c] = 3 + 6 * l + 3 + c
    w0p = np.ascontiguousarray(W0[perm]).astype(np.float16)
    w0rep = np.zeros((128, 256), np.float16)
    w0rep[0:39] = w0p
    w0rep[64:103] = w0p

    w2h = np.empty((128, 8), np.float16)
    w2h[:, 0:4] = W2[0:128].astype(np.float16)
    w2h[:, 4:8] = W2[128:256].astype(np.float16)
    b0t = np.ascontiguousarray(b0.reshape(2, 128).T).astype(f32)
    b1t = np.ascontiguousarray(b1.reshape(2, 128).T).astype(f32)
    b2t = np.broadcast_to(b2.astype(f32), (128, 4)).copy()

    q = np.arange(128)
    rp = q // 64
    s = q % 64
    zcpp = (NEAR + DELTA * s).astype(f32).reshape(128, 1).copy()

    # ltri: cols 0..127 exclusive, 128..255 inclusive
    # ltri[k=(rp',j), m=(rp,s)] = (rp'==rp) & (j < s)  /  (j <= s)
    kk = q
    krp = kk // 64
    kj = kk % 64
    same = (krp[:, None] == rp[None, :])
    ltri = np.zeros((128, 256), f32)
    ltri[:, 0:128] = (same & (kj[:, None] < s[None, :])).astype(f32)
    ltri[:, 128:256] = (same & (kj[:, None] <= s[None, :])).astype(f32)
    sel2 = (krp[:, None] == np.arange(2)[None, :]).astype(f32)
    ident = np.eye(128, dtype=f32)
    identh = np.eye(128, dtype=np.float16)

    # ray_of[J, rp] = 16*(J%128) + 2*(J//128) + rp
    J = np.arange(NBLK)
    ray_of = (16 * (J % 128))[:, None] + (2 * (J // 128))[:, None] + np.arange(2)[None, :]

    in_maps = []
    for core in range(NCORES):
        o = origins[core * BC : (core + 1) * BC].astype(f32)
        d = directions[core * BC : (core + 1) * BC].astype(f32)
        t = t_rand[core * BC : (core + 1) * BC].astype(f32)
        # host-side point-permutation transpose:
        # rays r = 16i + 2k + rp;  tnat[64*rp+s, 128*k+i] = t[r, s]
        tr = t.reshape(128, 8, 2, 64)          # [i, k, rp, s]
        tnat = np.ascontiguousarray(
            tr.transpose(2, 3, 1, 0).reshape(128, 1024)
        )
        # aexp[c, q, J] = o[ray_of[J, rp(q)], c]
        rays_qJ = ray_of[:, :].T[rp]  # [128, NBLK] -> rays_qJ[q, J] = ray_of[J, rp[q]]
        aexp = np.ascontiguousarray(o[rays_qJ].transpose(2, 0, 1))
        bexp = np.ascontiguousarray(d[rays_qJ].transpose(2, 0, 1))
        in_maps.append(
            {
                "tnat": tnat,
                "aexp": aexp,
                "bexp": bexp,
                "w0rep": w0rep,
                "w1": W1.astype(np.float16),
                "w2h": w2h,
                "b0t": b0t,
                "b1t": b1t,
                "b2t": b2t,
                "zcpp": zcpp,
                "ltri": ltri,
                "sel2": sel2,
                "ident": ident,
                "identh": identh,
            }
        )
    return in_maps, ray_of


def kernel(origins, directions, t_rand, W0, b0, W1, b1, W2, b2, near, far,
           **kw):
    assert int(near) == 2 and int(far) == 6
    from concourse.bass_utils import run_bass_kernel_spmd

    if "nc" not in _CACHE:
        _CACHE["nc"] = _build()
    nc = _CACHE["nc"]

    in_maps, ray_of = _host_prep(
        np.asarray(origins), np.asarray(directions), np.asarray(t_rand),
        np.asarray(W0), np.asarray(b0), np.asarray(W1), np.asarray(b1),
        np.asarray(W2), np.asarray(b2),
    )
    res = run_bass_kernel_spmd(
        nc, in_maps, core_ids=list(range(NCORES)), trace=PROFILE
    )
    _CACHE["last_results"] = res
    out = np.empty((B, 3), np.float32)
    for core in range(NCORES):
        oc = res.results[core]["out"].reshape(NGRP, 2, 128, 3)
        # group g holds blocks J = 128*g + i ; ray = 16*i + 2*g + rp
        for g in range(NGRP):
            for rpp in range(2):
                rays = core * BC + 16 * np.arange(128) + 2 * g + rpp
                out[rays] = oc[g, rpp]
    return out


# revision 41
# speedup vs baseline: 1.0594x; 1.0594x over previous
"""NeRF render kernel for 8 Trainium2 NeuronCores.

Data-parallel over rays: core k handles rays [2048*k, 2048*(k+1)).
Per core: positional encoding + 3-layer MLP (39->256->256->4) over
131072 points in feature-major layout (features on partitions, points
on the free dim), then alpha compositing via triangular-matrix matmul
cumulative sums.

Point permutation inside a core: t_rand rows are loaded as
[128 partitions = ray-group i (rays 16i..16i+15), 1024 = (k, s)] and
PE-transposed per 128-column chunk k0 so that partitions become
q = rp*64 + s (rp = ray parity) and columns J = 128*k0 + i denote the
ray pair (16i + 2*k0, 16i + 2*k0 + 1).  All downstream tiles keep that
column order; the host unscatters at the end.

Performance layout (v11):
- Staging is (block, feature64) so each 2-block pair is a contiguous
  [128, 128] unit; PE transposes move it to 64-aligned partition
  bands, and the L0 matmuls run row-tiled band pairs concurrently.
- The whole MLP is software-pipelined over 128 super-tiles: at
  iteration it the PE runs transposes(it), L0(it-1), L1(it-2),
  L2(it-3), so it never stalls on the ACT/DVE PSUM drains and the
  HAM clock stays warm (2.4 GHz).
- PSUM->SBUF relu drains split evenly ACT/DVE (h0 drains paired as
  single FD-1024 ops); sin-arg chain tensor_scalars on DVE/ACT,
  tensor_tensors on GPSIMD; cos features via cos(2a) = 1-2 sin^2(a)
  in fp16; t_rand is transposed host-side; compositing is deferred
  to an epilogue with all sigmoids batched before all exps (one
  activation-table swap each).
"""

import sys
import numpy as np

sys.path.insert(0, "/opt/trn_rl_repo")

S = 64
L = 6
NCORES = 8
B = 16384
BC = B // NCORES          # rays per core
NP = BC * S               # points per core
NBLK = NP // 128          # 1024 ray-pair blocks
NGRP = 8                  # groups of 128 blocks
HB = 64                   # blocks per half-group
NEAR, FAR = 2.0, 6.0
DELTA = (FAR - NEAR) / S
PI = float(np.pi)
TWO_PI = float(2.0 * np.pi)
INV2PI = float(np.float32(1.0 / (2.0 * np.pi)))
MAGIC = 12582912.0  # 1.5 * 2**23: float32 round-to-int trick
C1 = float(np.float32(2.0 * np.pi))
C2 = float(2.0 * np.pi - np.float64(np.float32(2.0 * np.pi)))

_CACHE = {}
PROFILE = False  # test harness sets True to collect an NTFF trace


def _split_waits(nc, mybir):
    """TRN2 allows one sem wait per instruction (two for EventSemaphore);
    this walrus build rejects over-limit instructions, so move excess waits
    onto chained NOPs on the same engine just before the instruction."""
    ctr = 0
    for fn in nc.m.functions:
        for bb in fn.blocks:
            changed = False
            out = []
            for inst in bb.instructions:
                si = inst.sync_info
                cap = 2 if isinstance(inst, mybir.InstEventSemaphore) else 1
                if si is not None and si.on_wait and len(si.on_wait) > cap:
                    waits = list(si.on_wait)
                    for w in waits[:-cap]:
                        nop = mybir.InstNoOp(
                            name=f"wsplit-{ctr}", ins=[], outs=[]
                        )
                        ctr += 1
                        nop.engine = inst.engine
                        nop.sync_info = mybir.SyncInfo(on_wait=[w], on_update=[])
                        nc.register_instruction(nop)
                        out.append(nop)
                    si.on_wait = waits[-cap:]
                    changed = True
                out.append(inst)
            if changed:
                bb.instructions = out
    return ctr


def _build():
    import concourse.bass as bass
    import concourse.mybir as mybir
    import concourse.tile as tile

    dt = mybir.dt
    AF = mybir.ActivationFunctionType
    OP = mybir.AluOpType
    F32 = dt.float32
    F32R = dt.float32r
    F16 = dt.float16

    nc = bass.Bass()

    # ---- DRAM I/O ----
    tnat_d = nc.dram_tensor("tnat", [128, 1024], F32, kind="ExternalInput")
    aexp_d = nc.dram_tensor("aexp", [3, 128, 1024], F32, kind="ExternalInput")
    bexp_d = nc.dram_tensor("bexp", [3, 128, 1024], F32, kind="ExternalInput")
    w0_d = nc.dram_tensor("w0rep", [128, 256], F16, kind="ExternalInput")
    w1_d = nc.dram_tensor("w1", [256, 256], F16, kind="ExternalInput")
    w2_d = nc.dram_tensor("w2h", [128, 8], F16, kind="ExternalInput")
    b0_d = nc.dram_tensor("b0t", [128, 2], F32, kind="ExternalInput")
    b1_d = nc.dram_tensor("b1t", [128, 2], F32, kind="ExternalInput")
    b2_d = nc.dram_tensor("b2t", [128, 4], F32, kind="ExternalInput")
    zcpp_d = nc.dram_tensor("zcpp", [128, 1], F32, kind="ExternalInput")
    ltri_d = nc.dram_tensor("ltri", [128, 256], F32, kind="ExternalInput")
    sel2_d = nc.dram_tensor("sel2", [128, 2], F32R, kind="ExternalInput")
    identh_d = nc.dram_tensor("identh", [128, 128], F16, kind="ExternalInput")
    out_d = nc.dram_tensor("out", [NGRP, 2, 384], F32, kind="ExternalOutput")

    with tile.TileContext(nc) as tc:
        with (
            tc.tile_pool(name="consts", bufs=1) as cpool,
            tc.tile_pool(name="tall", bufs=1) as tpool,
            tc.tile_pool(name="o2", bufs=8) as o2pool,
        ):
            # ---- load constants / weights ----
            tnat = cpool.tile([128, 1024], F32, tag="tnat")
            nc.sync.dma_start(tnat[:], tnat_d[:])
            aexs = []
            bexs = []
            for c in range(3):
                ae = cpool.tile([128, 1024], F32, tag=f"aex{c}", name=f"aex{c}")
                nc.sync.dma_start(ae[:], aexp_d[c])
                be = cpool.tile([128, 1024], F32, tag=f"bex{c}", name=f"bex{c}")
                nc.sync.dma_start(be[:], bexp_d[c])
                aexs.append(ae)
                bexs.append(be)
            zcpp = cpool.tile([128, 1], F32, tag="zcpp")
            nc.sync.dma_start(zcpp[:], zcpp_d[:])
            w0rep = cpool.tile([128, 256], F16, tag="w0rep")
            nc.scalar.dma_start(w0rep[:], w0_d[:])
            w1s0 = cpool.tile([128, 256], F16, tag="w1s0")
            nc.scalar.dma_start(w1s0[:], w1_d[0:128, :])
            w1s1 = cpool.tile([128, 256], F16, tag="w1s1")
            nc.scalar.dma_start(w1s1[:], w1_d[128:256, :])
            w2s = cpool.tile([128, 8], F16, tag="w2s")
            nc.scalar.dma_start(w2s[:], w2_d[:])
            b0t = cpool.tile([128, 2], F32, tag="b0t")
            nc.scalar.dma_start(b0t[:], b0_d[:])
            b1t = cpool.tile([128, 2], F32, tag="b1t")
            nc.scalar.dma_start(b1t[:], b1_d[:])
            b2t = cpool.tile([128, 4], F32, tag="b2t")
            nc.scalar.dma_start(b2t[:], b2_d[:])
            ltri = cpool.tile([128, 256], F32, tag="ltri")
            nc.scalar.dma_start(ltri[:], ltri_d[:])
            sel2 = cpool.tile([128, 2], F32R, tag="sel2")
            nc.scalar.dma_start(sel2[:], sel2_d[:])
            identh = cpool.tile([128, 128], F16, tag="identh")
            nc.scalar.dma_start(identh[:], identh_d[:])
            zerot = cpool.tile([128, 1], F32, tag="zerot")
            nc.vector.memset(zerot[:], 0.0)
            magict = cpool.tile([128, 1], F32, tag="magict")
            nc.vector.memset(magict[:], MAGIC)

            # ---- phase A: tnat arrives host-transposed; z + pts +
            # range reduction are all computed per half-group inside
            # chain_gen, so the serial head is just the DMA loads ----
            zt = tpool.tile([128, 1024], F32, tag="zt")
            pts3 = tpool.tile([128, 3072], F32, tag="pts3")
            pts3r = tpool.tile([128, 3072], F32, tag="pts3r")


            # ---- phase B + C: software-pipelined over 128 super-tiles ----
            # At iteration it the PE runs transposes(it), L0(it-1),
            # L1(it-2), L2(it-3): every stage consumes activations that
            # were drained a full iteration earlier, so the PE never
            # stalls on ACT/DVE drains and the HAM clock stays warm.
            NS = NBLK // 8            # 128 supers; super s = blocks 8s..8s+7
            NHG = 2 * NGRP            # 16 half-groups of 8 supers
            W = 3 * HB
            with (
                tc.tile_pool(name="scr", bufs=2) as scrpool,
                tc.tile_pool(name="kpool", bufs=2) as kpool,
                tc.tile_pool(name="sfp", bufs=2) as sfppool,
                tc.tile_pool(name="fs", bufs=4) as fspool,
                tc.tile_pool(name="h0s", bufs=4) as h0spool,
                tc.tile_pool(name="h1s", bufs=4) as h1spool,
                tc.tile_pool(name="cS", bufs=2) as cspool,
                tc.tile_pool(name="tpP", bufs=1, space="PSUM") as tp_pool,
                tc.tile_pool(name="h0P", bufs=2, space="PSUM") as h0_pool,
                tc.tile_pool(name="h1P", bufs=2, space="PSUM") as h1_pool,
                tc.tile_pool(name="oP", bufs=1, space="PSUM") as o_pool,
            ):
                sf_t = {}
                fs_t = {}
                h0_t = {}
                h1_t = {}
                og_t = {}

                def chain_gen(hg):
                    """Sin-arg chain + staging for one half-group, split
                    into 8 steps (one per pipeline iteration).

                    sa (f, j): rows 0:3 = x/2, 3:6 = x, 3(l+1)+c = wrapped
                    2^l x.  sf (j, f64): per block j a 64-slot group
                    [0:18 sin, 18:36 cos, 36:39 raw, 39:42 sin(x/2),
                    42:64 pad] so each 2-block pair is a contiguous
                    [128, 128] unit that a REGULAR matmul (chunk
                    stationary + FWL, identity moving) transposes into
                    64-aligned bands -- counting as PE activity (warm HAM).
                    """
                    jsl = slice(HB * hg, HB * (hg + 1))
                    csl = slice(3 * HB * hg, 3 * HB * (hg + 1))
                    nc.vector.tensor_scalar(
                        zt[:, jsl], tnat[:, jsl], DELTA, zcpp[:, 0:1],
                        op0=OP.mult, op1=OP.add,
                    )
                    for c in range(3):
                        pv = pts3.rearrange("p (j c) -> p c j", c=3)[:, c, jsl]
                        nc.vector.tensor_tensor(
                            pv, zt[:, jsl], bexs[c][:, jsl], op=OP.mult
                        )
                        nc.vector.tensor_tensor(
                            pv, pv, aexs[c][:, jsl], op=OP.add
                        )
                    kt = kpool.tile([128, W], F32, tag="kt", name="kt")
                    nc.vector.tensor_scalar(
                        kt[:], pts3[:, csl], INV2PI, MAGIC, op0=OP.mult, op1=OP.add
                    )
                    nc.vector.tensor_scalar(
                        kt[:], kt[:], MAGIC, -C1, op0=OP.subtract, op1=OP.mult
                    )
                    nc.vector.tensor_tensor(
                        pts3r[:, csl], pts3[:, csl], kt[:], op=OP.add
                    )
                    nc.vector.tensor_scalar(
                        pts3r[:, csl], pts3r[:, csl], PI, -PI, op0=OP.min, op1=OP.max
                    )
                    yield
                    sa = scrpool.tile([128, 7 * W], F32, tag="sa", name="sa")
                    p3rt = pts3r.rearrange(
                        "p (o j c) -> p o c j", o=NHG, c=3
                    )[:, hg, :, :]
                    sa7 = sa.rearrange("p (f c j) -> p f c j", f=7, c=3)
                    nc.vector.tensor_scalar(
                        sa7[:, 0], p3rt, 0.5, None, op0=OP.mult
                    )
                    nc.gpsimd.tensor_copy(sa7[:, 1], p3rt)
                    # r_l = 2 r_{l-1} - 2pi*round(r_{l-1}/pi)
                    for l in range(1, L):
                        prev = sa[:, l * W : (l + 1) * W]
                        cur = sa[:, (l + 1) * W : (l + 2) * W]
                        kb = kpool.tile([128, W], F32, tag="kb", name="kb")
                        if hg > 0:
                            nc.scalar.activation(
                                kb[:], prev, AF.Identity,
                                scale=2.0 * INV2PI, bias=magict[:, 0:1],
                            )
                        else:
                            nc.vector.tensor_scalar(
                                kb[:], prev, 2.0 * INV2PI, MAGIC,
                                op0=OP.mult, op1=OP.add,
                            )
                        nc.vector.tensor_scalar(
                            kb[:], kb[:], MAGIC, -PI,
                            op0=OP.subtract, op1=OP.mult,
                        )
                        if hg > 0:
                            nc.gpsimd.tensor_tensor(cur, prev, kb[:], op=OP.add)
                        else:
                            nc.vector.tensor_tensor(cur, prev, kb[:], op=OP.add)
                        nc.vector.tensor_scalar(
                            cur, cur, 2.0, None, op0=OP.mult
                        )
                        if l < L - 1:
                            yield
                    sf = sfppool.tile([128, 64 * HB], F16, tag="sf", name="sf")
                    sf_t[hg] = sf
                    sfv = sf.rearrange("p (j f) -> p j f", f=64)
                    saj = sa.rearrange("p (f j) -> p j f", j=HB)
                    nc.scalar.activation(
                        sfv[:, :, 39:42], saj[:, :, 0:3], AF.Sin
                    )
                    yield
                    nc.scalar.activation(
                        sfv[:, :, 0:18], saj[:, :, 3:21], AF.Sin
                    )
                    # cos_l = 1 - 2 sin(2^{l-1} x)^2   (fp16, DVE)
                    for l in range(L):
                        if l == 2:
                            yield
                        src = (sfv[:, :, 39:42] if l == 0
                               else sfv[:, :, 3 * (l - 1) : 3 * l])
                        kb16 = kpool.tile([128, W], F16, tag="kb16", name="kb16")
                        kb16v = kb16.rearrange("p (j c) -> p j c", c=3)
                        nc.vector.tensor_tensor(kb16v, src, src, op=OP.mult)
                        nc.vector.tensor_scalar(
                            sfv[:, :, 18 + 3 * l : 21 + 3 * l],
                            kb16v, -2.0, 1.0, op0=OP.mult, op1=OP.add,
                        )
                    nc.vector.tensor_copy(
                        sfv[:, :, 36:39],
                        pts3.rearrange("p (j c) -> p j c", c=3)[
                            :, HB * hg : HB * (hg + 1), :
                        ],
                    )
                    yield

                def stage_T(s):
                    sf = sf_t[s // 8]
                    stl = s % 8
                    tp = tp_pool.tile([128, 512], F16, tag="tp", name="tp")
                    for k in range(4):
                        jj = 4 * stl + k
                        nc.tensor.transpose(
                            tp[:, 128 * k : 128 * (k + 1)],
                            sf[:, 128 * jj : 128 * (jj + 1)],
                            identh[:],
                        )
                    fs = fspool.tile([128, 512], F16, tag="fs", name="fs")
                    nc.scalar.activation(fs[:], tp[:], AF.Copy)
                    fs_t[s] = fs

                def stage_L0(s):
                    # L0 row-tiled band pairs; per hidden-half the two
                    # band outputs share one [128, 1024] PSUM tile so the
                    # relu+bias drain is a single FD-1024 op (uniform bias)
                    fs = fs_t.pop(s)
                    h0ss = [
                        h0spool.tile([128, 1024], F16, tag="h0s", name=f"h0s{s}_{h}")
                        for h in range(2)
                    ]
                    for h in range(2):
                        h0p = h0_pool.tile([128, 1024], F32, tag="h0p", name="h0p")
                        for x in range(2):
                            lo = 64 * x
                            nc.tensor.matmul(
                                h0p[:, 512 * x : 512 * (x + 1)],
                                w0rep[lo : lo + 39, 128 * h : 128 * (h + 1)],
                                fs[lo : lo + 39, :],
                            )
                        if h == 0:
                            nc.scalar.activation(
                                h0ss[0][:], h0p[:], AF.Relu, bias=b0t[:, 0:1]
                            )
                        else:
                            nc.vector.tensor_scalar(
                                h0ss[1][:], h0p[:], b0t[:, 1:2], 0.0,
                                op0=OP.add, op1=OP.max,
                            )
                    h0_t[s] = h0ss

                def stage_L1(s):
                    h0ss = h0_t.pop(s)
                    h1ss = [
                        h1spool.tile([128, 1024], F16, tag="h1s", name=f"h1s{s}_{g}")
                        for g in range(2)
                    ]
                    for x in range(2):
                        for gh in range(2):
                            h1p = h1_pool.tile([128, 512], F32, tag="h1p", name="h1p")
                            nc.tensor.matmul(
                                h1p[:],
                                w1s0[:, 128 * gh : 128 * (gh + 1)],
                                h0ss[0][:, 512 * x : 512 * (x + 1)],
                                start=True,
                                stop=False,
                            )
                            nc.tensor.matmul(
                                h1p[:],
                                w1s1[:, 128 * gh : 128 * (gh + 1)],
                                h0ss[1][:, 512 * x : 512 * (x + 1)],
                                start=False,
                                stop=True,
                            )
                            dst = h1ss[gh][:, 512 * x : 512 * (x + 1)]
                            if gh == 0:
                                nc.scalar.activation(
                                    dst, h1p[:], AF.Relu, bias=b1t[:, 0:1]
                                )
                            else:
                                nc.vector.tensor_scalar(
                                    dst, h1p[:], b1t[:, 1:2], 0.0,
                                    op0=OP.add, op1=OP.max,
                                )
                    h1_t[s] = h1ss

                def stage_L2(s):
                    h1ss = h1_t.pop(s)
                    g = s // 16
                    if s % 16 == 0:
                        og_t[g] = o_pool.tile([128, 512], F32, tag="og", name="og")
                    og = og_t[g]
                    # band x holds blocks of parity x
                    for x in range(2):
                        for jp in range(4):
                            jj = 8 * (s % 16) + 2 * jp + x
                            nc.tensor.matmul(
                                og[:, 4 * jj : 4 * (jj + 1)],
                                h1ss[0][:, 512 * x + 128 * jp : 512 * x + 128 * (jp + 1)],
                                w2s[:, 0:4],
                                start=True,
                                stop=False,
                            )
                            nc.tensor.matmul(
                                og[:, 4 * jj : 4 * (jj + 1)],
                                h1ss[1][:, 512 * x + 128 * jp : 512 * x + 128 * (jp + 1)],
                                w2s[:, 4:8],
                                start=False,
                                stop=True,
                            )
                    if s % 16 == 15:
                        emit_groupC(g)

                o2_t = {}

                def emit_groupC(g):
                    # og -> o2 drain only (no table-switching ACT funcs);
                    # the compositing itself is deferred to the epilogue
                    og = og_t.pop(g)
                    o2 = o2pool.tile([128, 512], F32, tag="o2", name="o2")
                    o2_t[g] = o2
                    ogv = og.rearrange("p (j c) -> p j c", c=4)
                    o2v = o2.rearrange("p (j c) -> p j c", c=4)
                    nc.scalar.activation(
                        o2v[:, :, 0], ogv[:, :, 0], AF.Identity, bias=b2t[:, 0:1]
                    )
                    nc.vector.tensor_scalar(
                        o2v[:, :, 1], ogv[:, :, 1], b2t[:, 1:2], None, op0=OP.add
                    )
                    nc.scalar.activation(
                        o2v[:, :, 2], ogv[:, :, 2], AF.Identity, bias=b2t[:, 2:3]
                    )
                    nc.vector.tensor_scalar(
                        o2v[:, :, 3], ogv[:, :, 3], b2t[:, 3:4], 0.0,
                        op0=OP.add, op1=OP.max,
                    )

                e_t = {}

                def emit_sigmoid(g):
                    o2v = o2_t[g].rearrange("p (j c) -> p j c", c=4)
                    e = cspool.tile([128, 384], F32, tag="e", name="e", bufs=NGRP)
                    e_t[g] = e
                    nc.scalar.activation(
                        e.rearrange("p (j c) -> p j c", c=3),
                        o2v[:, :, 0:3],
                        AF.Sigmoid,
                    )

                def emit_compositing(g):
                    o2 = o2_t.pop(g)
                    o2v = o2.rearrange("p (j c) -> p j c", c=4)
                    e = e_t.pop(g)
                    # scans: exclusive & inclusive cumsum of sigma over s
                    ct = h1_pool.tile([128, 512], F32, tag="h1p", name="ct")
                    sig = o2v[:, :, 3]
                    nc.tensor.matmul(ct[:, 0:128], ltri[:, 0:128], sig)
                    nc.tensor.matmul(ct[:, 128:256], ltri[:, 128:256], sig)
                    texin = cspool.tile([128, 256], F32, tag="texin", name="texin")
                    nc.scalar.activation(texin[:], ct[:, 0:256], AF.Exp, scale=-DELTA)
                    wt = cspool.tile([128, 128], F32, tag="wt", name="wt")
                    nc.vector.tensor_tensor(
                        wt[:], texin[:, 0:128], texin[:, 128:256], op=OP.subtract
                    )
                    wr = cspool.tile([128, 384], F32R, tag="wr", name="wr")
                    nc.vector.tensor_tensor(
                        wr.rearrange("p (j c) -> p j c", c=3),
                        e.rearrange("p (j c) -> p j c", c=3),
                        wt.unsqueeze(2).broadcast_to([128, 128, 3]),
                        op=OP.mult,
                    )
                    # final per-ray-parity sum into spare cols of ct's bank
                    rp_ = ct[0:2, 128:512]
                    nc.tensor.matmul(rp_, sel2[:], wr[:])
                    outs = cspool.tile([2, 384], F32, tag="outs", name="outs")
                    nc.vector.tensor_copy(outs[:], rp_)
                    nc.sync.dma_start(out_d[g], outs[:])

                gen = chain_gen(0)
                for _ in gen:
                    pass
                gens = {}
                for it in range(NS + 3):
                    if it < NS:
                        hg_next = it // 8 + 1
                        if hg_next < NHG:
                            if it % 8 == 0:
                                gens[hg_next] = chain_gen(hg_next)
                            next(gens[hg_next], None)
                        stage_T(it)
                    if 1 <= it <= NS:
                        stage_L0(it - 1)
                    if 2 <= it <= NS + 1:
                        stage_L1(it - 2)
                    if 3 <= it <= NS + 2:
                        stage_L2(it - 3)
                tc.no_sync_barrier()
                for g in range(NGRP):
                    emit_sigmoid(g)
                for g in range(NGRP):
                    emit_compositing(g)

    _split_waits(nc, mybir)
    return nc


def _host_prep(origins, directions, t_rand, W0, b0, W1, b1, W2, b2):
    """Build per-core input maps (all numpy, cheap)."""
    f32 = np.float32
    # F-row order: rows 3l+c = sin freq l coord c; 18+3l+c = cos; 36..38 pts
    perm = np.zeros(39, np.int64)
    perm[36:39] = (0, 1, 2)
    for l in range(L):
        for c in range(3):
            perm[3 * l + c] = 3 + 6 * l + c
            perm[18 + 3 * l + c] = 3 + 6 * l + 3 + c
    w0p = np.ascontiguousarray(W0[perm]).astype(np.float16)
    w0rep = np.zeros((128, 256), np.float16)
    w0rep[0:39] = w0p
    w0rep[64:103] = w0p

    w2h = np.empty((128, 8), np.float16)
    w2h[:, 0:4] = W2[0:128].astype(np.float16)
    w2h[:, 4:8] = W2[128:256].astype(np.float16)
    b0t = np.ascontiguousarray(b0.reshape(2, 128).T).astype(f32)
    b1t = np.ascontiguousarray(b1.reshape(2, 128).T).astype(f32)
    b2t = np.broadcast_to(b2.astype(f32), (128, 4)).copy()

    q = np.arange(128)
    rp = q // 64
    s = q % 64
    zcpp = (NEAR + DELTA * s).astype(f32).reshape(128, 1).copy()

    # ltri: cols 0..127 exclusive, 128..255 inclusive
    # ltri[k=(rp',j), m=(rp,s)] = (rp'==rp) & (j < s)  /  (j <= s)
    kk = q
    krp = kk // 64
    kj = kk % 64
    same = (krp[:, None] == rp[None, :])
    ltri = np.zeros((128, 256), f32)
    ltri[:, 0:128] = (same & (kj[:, None] < s[None, :])).astype(f32)
    ltri[:, 128:256] = (same & (kj[:, None] <= s[None, :])).astype(f32)
    sel2 = (krp[:, None] == np.arange(2)[None, :]).astype(f32)
    ident = np.eye(128, dtype=f32)
    identh = np.eye(128, dtype=np.float16)

    # ray_of[J, rp] = 16*(J%128) + 2*(J//128) + rp
    J = np.arange(NBLK)
    ray_of = (16 * (J % 128))[:, None] + (2 * (J // 128))[:, None] + np.arange(2)[None, :]

    in_maps = []
    for core in range(NCORES):
        o = origins[core * BC : (core + 1) * BC].astype(f32)
        d = directions[core * BC : (core + 1) * BC].astype(f32)
        t = t_rand[core * BC : (core + 1) * BC].astype(f32)
        # host-side point-permutation transpose:
        # rays r = 16i + 2k + rp;  tnat[64*rp+s, 128*k+i] = t[r, s]
        tr = t.reshape(128, 8, 2, 64)          # [i, k, rp, s]
        tnat = np.ascontiguousarray(
            tr.transpose(2, 3, 1, 0).reshape(128, 1024)
        )
        # aexp[c, q, J] = o[ray_of[J, rp(q)], c]
        rays_qJ = ray_of[:, :].T[rp]  # [128, NBLK] -> rays_qJ[q, J] = ray_of[J, rp[q]]
        aexp = np.ascontiguousarray(o[rays_qJ].transpose(2, 0, 1))
        bexp = np.ascontiguousarray(d[rays_qJ].transpose(2, 0, 1))
        in_maps.append(
            {
                "tnat": tnat,
                "aexp": aexp,
                "bexp": bexp,
                "w0rep": w0rep,
                "w1": W1.astype(np.float16),
                "w2h": w2h,
                "b0t": b0t,
                "b1t": b1t,
                "b2t": b2t,
                "zcpp": zcpp,
                "ltri": ltri,
                "sel2": sel2,
                "identh": identh,
            }
        )
    return in_maps, ray_of


def kernel(origins, directions, t_rand, W0, b0, W1, b1, W2, b2, near, far,
           **kw):
    assert int(near) == 2 and int(far) == 6
    from concourse.bass_utils import run_bass_kernel_spmd

    if "nc" not in _CACHE:
        _CACHE["nc"] = _build()
    nc = _CACHE["nc"]

    in_maps, ray_of = _host_prep(
        np.asarray(origins), np.asarray(directions), np.asarray(t_rand),
        np.asarray(W0), np.asarray(b0), np.asarray(W1), np.asarray(b1),
        np.asarray(W2), np.asarray(b2),
    )
    res = run_bass_kernel_spmd(
        nc, in_maps, core_ids=list(range(NCORES)), trace=PROFILE
    )
    _CACHE["last_results"] = res
    out = np.empty((B, 3), np.float32)
    for core in range(NCORES):
        oc = res.results[core]["out"].reshape(NGRP, 2, 128, 3)
        # group g holds blocks J = 128*g + i ; ray = 16*i + 2*g + rp
        for g in range(NGRP):
            for rpp in range(2):
                rays = core * BC + 16 * np.arange(128) + 2 * g + rpp
                out[rays] = oc[g, rpp]
    return out
